# revision 33
# baseline (speedup 1.0000x reference)
"""nn_CrossAttention TRN2 kernel v4 — 8-core SPMD Bass/Tile, query-sharded.

Sharding: core p -> batch b = p//2, query-half g = p%2.
Each core: its 2048 queries against all 4096 keys -> softmax completes
locally (no partial-D exchange).

Host-side prep (layout/dtype only + weight folding):
  xT, tT   channel-major bf16 transposes of x / t (kills all input
           PE-transposes and casts; halves input DMA bytes)
  A        Wq @ Wk.T (f32 fold, bf16 cast) -> scores = (t A) x^T, so the
           k-projection disappears and xT is the score stationary operand

Per-core dataflow:
  qAT      = A^T tT   channel-major bf16 [384, 2048]
  v        = x Wv'    row-major bf16 [keys, 384]; Wv' is Wv with columns
           permuted per-core to [peer's channel half | own half] so the
           exchange slicing is rank-INDEPENDENT (SPMD single program)
  s->e->o  per chunk (512 queries) x n (128 keys): scores (3 MM), exp
           (ACT), o^T accumulation (3 MM), software-pipelined ACROSS chunk
           boundaries so the PE stream never drains; the softmax
           denominator accumulates on DVE (no PE ones-matmul) and is
           partition-reduced once per chunk on GpSimd
  normalize per chunk: reciprocal -> partition-broadcast -> 3 DVE muls
           (PSUM -> bf16 SBUF), all-f32 denominators
  repeat pipelining: the next pass's input DMAs + qA/v projections are
           emitted between a pass's attention and its tail, so the PE
           fills the final-exchange latency with useful work and HAM
           stays warm into the projection
  AllGather per T: pairwise, ships only the peer-destined 192 channels
           (half the ReduceScatter wire bytes, pure data movement, no
           partial-D exchange). The received block and the locally-kept
           half are placed into the channel-major zbuf via exact 0/1
           mask multiplies (masks are host inputs = the core's rank bit),
           keeping every instruction rank-independent.
  zbuf     channel-major [192, 4096]: the "transpose(1,2).reshape"
           permutation becomes contiguous rows
  out proj reads zbuf rows, PE-transposes, Wp matmul; bias via DVE add
"""
from contextlib import ExitStack

import numpy as np
import ml_dtypes

import concourse.bass as bass
import concourse.bass_isa as bass_isa
import concourse.tile as tile
from concourse import bacc, mybir
from concourse.bass_utils import run_bass_kernel_spmd
from concourse.masks import make_identity

F32 = mybir.dt.float32
BF16 = mybir.dt.bfloat16
EXP = mybir.ActivationFunctionType.Exp

B, N, TN, C = 4, 4096, 4096, 384
QS = TN // 2           # queries per core
CH = C // 2            # o^T channels per core after exchange
SCALE = (C // 8) ** -0.5
N_CORES = 8
GROUPS = [[0, 1], [2, 3], [4, 5], [6, 7]]
TAIL_LOCAL = False     # compute the last QR queries of BOTH ranks locally
QR = 256               # redundantly-computed peer queries (tail chunk)
QE = QS + QR           # query columns per core incl. peer tail
# (start, width) of attention chunks over the QE query columns; the last
# chunk = own tail QR + peer tail QR, computed locally on BOTH cores so no
# collective sits on the critical tail, and the preceding exchanges hide
# under the later chunks' compute
CHUNKS = [(0, 512), (512, 512), (1024, 512), (1536, QS - 1536 - QR),
          (QS - QR, 2 * QR)]


def build(repeat=1, stop_after=None, sim_cc=False, tail_local=TAIL_LOCAL):
    nc = bacc.Bacc("TRN2", target_bir_lowering=False, debug=False,
                   num_devices=N_CORES)
    nc._sim_cc = sim_cc
    nc._tail_local = tail_local
    xT_d = nc.dram_tensor("xT", [C, N], BF16, kind="ExternalInput").ap()
    tT_d = nc.dram_tensor("tT", [C, QE], BF16, kind="ExternalInput").ap()
    w_d = {n: nc.dram_tensor(n, [C, C], BF16, kind="ExternalInput").ap()
           for n in ("A", "Wv", "Wp")}
    bp_d = nc.dram_tensor("bp", [1, C], F32, kind="ExternalInput").ap()
    sel_d = nc.dram_tensor("sel", [1, 1024], F32, kind="ExternalInput").ap()
    out_d = nc.dram_tensor("out", [QS, C], F32, kind="ExternalOutput").ap()

    with tile.TileContext(nc) as tc:
        _kernel_body(nc, tc, xT_d, tT_d, w_d, bp_d, sel_d, out_d, repeat,
                     stop_after)
    nc.compile()
    return nc


def _kernel_body(nc, tc, xT_d, tT_d, w_d, bp_d, sel_d, out_d, repeat,
                 stop_after):
    with ExitStack() as ctx:
        consts = ctx.enter_context(tc.tile_pool(name="consts", bufs=1))
        dram = ctx.enter_context(tc.tile_pool(name="dram", bufs=1, space="DRAM"))

        ident = consts.tile([128, 128], BF16)
        make_identity(nc, ident)

        # weights / bias: loaded once (not in the repeat loop)
        w_sb = {}
        for name in ("A", "Wv", "Wp"):
            cw = consts.tile([128, 3 * C], BF16, name=f"{name}_sb",
                             tag=f"{name}_sb")
            for dc in range(3):
                nc.sync.dma_start(cw[:, dc * C:(dc + 1) * C],
                                  w_d[name][dc * 128:(dc + 1) * 128, :])
            w_sb[name] = cw
        bp_sb = consts.tile([1, C], F32, name="bp_sb", tag="bp_sb")
        nc.sync.dma_start(bp_sb[:], bp_d[:])
        bias_bc = consts.tile([128, C], F32, name="bias_bc", tag="bias_bc")
        nc.gpsimd.partition_broadcast(bias_bc[:], bp_sb[:])

        # rank-bit masks: sel row0 = (own rank is group-rank 0) as 0/1,
        # row1 = the complement.  Broadcast to full tiles once.
        sel_sb = consts.tile([1, 1024], F32, name="sel_sb", tag="sel_sb")
        nc.sync.dma_start(sel_sb[:], sel_d[:])
        s_bc = consts.tile([128, 512], F32, name="s_bc", tag="s_bc")
        nc.gpsimd.partition_broadcast(s_bc[:], sel_sb[:, 0:512])
        m_bc = consts.tile([128, 512], F32, name="m_bc", tag="m_bc")
        nc.gpsimd.partition_broadcast(m_bc[:], sel_sb[:, 512:1024])

        def wch(name, dc, cc=None):
            if cc is None:
                return w_sb[name][:, dc * C:(dc + 1) * C]
            return w_sb[name][:, dc * C + cc * 128: dc * C + (cc + 1) * 128]

        chunks = CHUNKS if getattr(nc, "_tail_local", True) else \
            [(0, 512), (512, 512), (1024, 512), (1536, 512)]
        args = (nc, tc, xT_d, tT_d, out_d, ident, wch, bias_bc, s_bc, m_bc,
                dram, chunks)
        st = _stage_load(*args)
        for rep in range(repeat):
            mid = _stage_attn(st, rep, stop_after, *args)
            if stop_after == "load" or stop_after == "att2":
                if rep + 1 < repeat:
                    st = _stage_load(*args)
                continue
            # next rep's loads + qA + v emit BEFORE this rep's tail, so the
            # PE fills the final-exchange gap with useful work (and HAM
            # stays warm into the projection)
            if rep + 1 < repeat:
                st = _stage_load(*args)
            _stage_tail(st if rep + 1 >= repeat else None, mid, rep,
                        *args)


def _stage_load(nc, tc, xT_d, tT_d, out_d, ident, wch, bias_bc, s_bc,
                m_bc, dram, chunks):
    NK = N // 128
    attin = tc.alloc_tile_pool(name="attin", bufs=1)
    # ---- input loads: tT first (unblocks qA), column-chunked ----
    tT = [attin.tile([128, QE], BF16, name=f"tT{dc}", tag=f"tT{dc}")
          for dc in range(3)]
    for q0, w in chunks:
        for dc in range(3):
            nc.sync.dma_start(tT[dc][:, q0:q0 + w],
                              tT_d[dc * 128:(dc + 1) * 128, q0:q0 + w])
    xT = [attin.tile([128, N], BF16, name=f"xT{cc}", tag=f"xT{cc}")
          for cc in range(3)]
    for nx in range(4):
        xsl = slice(nx * (N // 4), (nx + 1) * (N // 4))
        for cc in range(3):
            nc.sync.dma_start(xT[cc][:, xsl],
                              xT_d[cc * 128:(cc + 1) * 128, xsl])

    # ---- qAT = A^T tT ----
    qAT = [attin.tile([128, QE], BF16, name=f"qAT{cc}", tag=f"qAT{cc}")
           for cc in range(3)]
    with tc.tile_pool(name="qpsum", bufs=2, space="PSUM") as qpsum:
        for q0, w in chunks:
            for cc in range(3):
                ps = qpsum.tile([128, 512], F32, name="qps", tag="qps")
                for dc in range(3):
                    nc.tensor.matmul(
                        ps[:, 0:w], wch("A", dc, cc),
                        tT[dc][:, q0:q0 + w],
                        start=(dc == 0), stop=(dc == 2))
                nc.scalar.copy(qAT[cc][:, q0:q0 + w], ps[:, 0:w])

    # ---- v = x Wv ----
    v_all = attin.tile([128, NK * C], BF16, name="v_all", tag="v_all")
    with tc.tile_pool(name="vpsum", bufs=3, space="PSUM") as vpsum:
        for n in range(NK):
            ps = vpsum.tile([128, C], F32, name="vps", tag="vps")
            for dc in range(3):
                nc.tensor.matmul(
                    ps[:], xT[dc][:, n * 128:(n + 1) * 128],
                    wch("Wv", dc), start=(dc == 0), stop=(dc == 2))
            nc.vector.tensor_copy(v_all[:, n * C:(n + 1) * C], ps[:])
    return (attin, xT, tT, qAT, v_all)


def _stage_attn(st, rep, stop_after, nc, tc, xT_d, tT_d, out_d, ident, wch,
                bias_bc, s_bc, m_bc, dram, chunks):
    NK = N // 128
    attin, xT, tT, qAT, v_all = st
    if stop_after == "load":
        attin.release()
        return None
    zbuf = dram.tile([QS, C], BF16, name=f"zbuf{rep}", tag="zbuf")
    zchan = zbuf[:].rearrange("a b -> (a b)").rearrange("(c t) -> c t",
                                                        t=N)
    sbufs = [dram.tile([CH, w], BF16, name=f"sb{rep}_{T}", tag=f"sb{T}")
             for T, (q0, w) in enumerate(chunks) if _ships(nc, T, chunks)]
    rbufs = [dram.tile([2, CH, w], BF16, name=f"rb{rep}_{T}", tag=f"rb{T}")
             for T, (q0, w) in enumerate(chunks) if _ships(nc, T, chunks)]

    npool = tc.alloc_tile_pool(name="npool", bufs=2, side="right")
    spsum = tc.alloc_tile_pool(name="spsum", bufs=2, space="PSUM")
    opsum = tc.alloc_tile_pool(name="opsum", bufs=1, space="PSUM")
    epool = tc.alloc_tile_pool(name="epool", bufs=8, side="right")
    if True:
        prev = None

        def flush_prev():
            nonlocal prev
            if prev is not None:
                e_p, n_p, ops_p, w_p = prev
                for cc in range(3):
                    nc.tensor.matmul(
                        ops_p[cc][:, 0:w_p],
                        v_all[:, n_p * C + cc * 128:
                              n_p * C + (cc + 1) * 128],
                        e_p[:, 0:w_p],
                        start=(n_p == 0), stop=(n_p == NK - 1))
                prev = None

        o_sets, e_sums = [], []
        for T, (q0, w) in enumerate(chunks):
            o_ps = [opsum.tile([128, 512], F32, name=f"ops{cc}",
                               tag=f"ops{cc}") for cc in range(3)]
            e_sum = epool.tile([128, 512], F32, name="e_sum", tag="e_sum")
            o_sets.append(o_ps)
            e_sums.append(e_sum)
            for n in range(NK):
                s_ps = spsum.tile([128, 512], F32, name="sps", tag="sps")
                for cc in range(3):
                    nc.tensor.matmul(
                        s_ps[:, 0:w], xT[cc][:, n * 128:(n + 1) * 128],
                        qAT[cc][:, q0:q0 + w],
                        start=(cc == 0), stop=(cc == 2))
                e_t = epool.tile([128, 512], BF16, name="e_t", tag="e_t")
                nc.scalar.activation(e_t[:, 0:w], s_ps[:, 0:w], EXP,
                                     scale=SCALE)
                # denominator accumulates on DVE (PE freed of the
                # ones-matmul); partition-reduced once per chunk
                if n == 0:
                    nc.vector.tensor_copy(e_sum[:, 0:w], e_t[:, 0:w])
                else:
                    nc.vector.tensor_add(e_sum[:, 0:w], e_sum[:, 0:w],
                                         e_t[:, 0:w])
                flush_prev()
                prev = (e_t, n, o_ps, w)
                # the n==0 flush above issued the previous chunk's last
                # o-matmuls -> ship it now so the exchange overlaps this
                # chunk's compute
                if n == 0 and T > 0:
                    _norm_ship(nc, T - 1, o_sets[T - 1], e_sums[T - 1],
                               npool, sbufs, rbufs, zchan, s_bc, m_bc,
                               chunks)
        flush_prev()
    attin.release()
    return (npool, spsum, opsum, epool, o_sets, e_sums, sbufs, rbufs, zbuf,
            zchan)


def _stage_tail(next_st, mid, rep, nc, tc, xT_d, tT_d, out_d, ident, wch,
                bias_bc, s_bc, m_bc, dram, chunks):
    if mid is None:
        return
    (npool, spsum, opsum, epool, o_sets, e_sums, sbufs, rbufs, zbuf,
     zchan) = mid
    _norm_ship(nc, len(chunks) - 1, o_sets[-1], e_sums[-1],
               npool, sbufs, rbufs, zchan, s_bc, m_bc, chunks)
    opsum.release()
    spsum.release()
    epool.release()
    npool.release()

    # ---- output projection over own 2048 rows ----
    NF = QS // 128
    with tc.tile_pool(name="fpool", bufs=4) as fpool, \
         tc.tile_pool(name="fpsum", bufs=3, space="PSUM") as fpsum, \
         tc.tile_pool(name="ftpsum", bufs=3, space="PSUM") as ftpsum:
        def f_load(it):
            r_t = fpool.tile([128, C], BF16, name="r_t", tag="r_t")
            nc.sync.dma_start(r_t[:], zbuf[it * 128:(it + 1) * 128, :])
            bank = ftpsum.tile([128, C], BF16, name="f_tr", tag="f_tr")
            for jc in range(3):
                nc.tensor.transpose(bank[:, jc * 128:(jc + 1) * 128],
                                    r_t[:, jc * 128:(jc + 1) * 128], ident[:])
            op_ch = fpool.tile([128, C], BF16, name="op_ch", tag="op_ch")
            nc.scalar.copy(op_ch[:], bank[:])
            return op_ch

        op_q = [f_load(0), f_load(1)]
        for it in range(NF):
            op_ch = op_q.pop(0)
            if it + 2 < NF:
                op_q.append(f_load(it + 2))
            out_ps = fpsum.tile([128, C], F32, name="out_ps", tag="out_ps")
            for jc in range(3):
                nc.tensor.matmul(out_ps[:], op_ch[:, jc * 128:(jc + 1) * 128],
                                 wch("Wp", jc), start=(jc == 0),
                                 stop=(jc == 2))
            o_t = fpool.tile([128, C], F32, name="o_t", tag="o_t")
            nc.vector.tensor_add(o_t[:], out_ps[:], bias_bc[:])
            nc.sync.dma_start(out_d[it * 128:(it + 1) * 128, :], o_t[:])


def _ships(nc, T, chunks):
    return not (getattr(nc, "_tail_local", True) and T == len(chunks) - 1)


def _norm_ship(nc, T, o_ps, e_sum, npool, sbufs, rbufs, zchan, s_bc, m_bc,
               chunks):
    q0, w = chunks[T]
    last = not _ships(nc, T, chunks)
    e_red = npool.tile([128, 512], F32, name="e_red", tag="e_red")
    nc.gpsimd.partition_all_reduce(e_red[:, 0:w], e_sum[:, 0:w], 128,
                                   bass_isa.ReduceOp.add)
    rec = npool.tile([1, 512], F32, name="rec", tag="rec")
    nc.vector.reciprocal(rec[:, 0:w], e_red[0:1, 0:w])
    rec_b = npool.tile([128, 512], F32, name="rec_b", tag="rec_b")
    nc.gpsimd.partition_broadcast(rec_b[:, 0:w], rec[:, 0:w])
    oTn = [npool.tile([128, 512], BF16, name=f"oTn{cc}", tag=f"oTn{cc}")
           for cc in range(3)]
    for cc in range(3):
        nc.vector.tensor_mul(oTn[cc][:, 0:w], o_ps[cc][:, 0:w],
                             rec_b[:, 0:w])
    coef = (s_bc, m_bc)
    if not last:
        # ship peer-destined half (oTn rows 0..192 by Wv-perm)
        sb = sbufs[T]
        nc.sync.dma_start(sb[0:128, :], oTn[0][:, 0:w])
        nc.sync.dma_start(sb[128:CH, :], oTn[1][0:64, 0:w])
        if getattr(nc, "_sim_cc", False):
            for j in range(2):
                nc.sync.dma_start(rbufs[T][j], sb[:])
        else:
            nc.gpsimd.collective_compute(
                "AllGather", mybir.AluOpType.bypass,
                replica_groups=GROUPS,
                ins=[sb[:].opt()], outs=[rbufs[T][:].opt()])
        # place own half + received peer half into zchan at their GLOBAL
        # query columns, selected by exact 0/1 rank masks:
        #   col-block j (global cols j*QS+q0) =
        #       own * (j == my rank) + rb[j] * (j == peer rank)
        rbt = [npool.tile([128, 512], BF16, name=f"rba{j}", tag=f"rba{j}")
               for j in range(2)]
        rbb = [npool.tile([128, 512], BF16, name=f"rbb{j}", tag=f"rbb{j}")
               for j in range(2)]
        for j in range(2):
            # top 64 received rows land on partitions 64..128 so every DVE
            # op below is partition-aligned with the own rows of oTn[1]
            nc.sync.dma_start(rbt[j][64:128, 0:w], rbufs[T][j, 0:64, :])
            nc.sync.dma_start(rbb[j][:, 0:w], rbufs[T][j, 64:CH, :])
        own_a = oTn[1][64:128, 0:w]
        own_b = oTn[2][:, 0:w]
        for j in range(2):
            za = npool.tile([128, 512], BF16, name=f"za{j}", tag=f"za{j}")
            zb = npool.tile([128, 512], BF16, name=f"zb{j}", tag=f"zb{j}")
            ta = npool.tile([128, 512], BF16, name=f"ta{j}", tag=f"ta{j}")
            tb = npool.tile([128, 512], BF16, name=f"tb{j}", tag=f"tb{j}")
            nc.vector.tensor_mul(za[64:128, 0:w], own_a,
                                 coef[j][64:128, 0:w])
            nc.vector.tensor_mul(zb[:, 0:w], own_b, coef[j][:, 0:w])
            nc.vector.tensor_mul(ta[64:128, 0:w], rbt[j][64:128, 0:w],
                                 coef[1 - j][64:128, 0:w])
            nc.vector.tensor_mul(tb[:, 0:w], rbb[j][:, 0:w],
                                 coef[1 - j][:, 0:w])
            nc.vector.tensor_add(za[64:128, 0:w], za[64:128, 0:w],
                                 ta[64:128, 0:w])
            nc.vector.tensor_add(zb[:, 0:w], zb[:, 0:w], tb[:, 0:w])
            csl = slice(j * QS + q0, j * QS + q0 + w)
            nc.sync.dma_start(zchan[0:64, csl], za[64:128, 0:w])
            nc.sync.dma_start(zchan[64:CH, csl], zb[:, 0:w])
    else:
        # tail chunk, fully local on both cores and packed in RANK order by
        # the host (rank 0's tail queries then rank 1's, identical on both
        # cores), so the z writes are fixed slices -- no masks at all
        for j in range(2):
            csl = slice(j * QS + q0, j * QS + q0 + QR)
            nc.sync.dma_start(zchan[0:64, csl],
                              oTn[1][64:128, j * QR:(j + 1) * QR])
            nc.sync.dma_start(zchan[64:CH, csl],
                              oTn[2][:, j * QR:(j + 1) * QR])


def make_in_maps(inputs):
    bf = ml_dtypes.bfloat16
    x = np.asarray(inputs["x"], np.float32)
    t = np.asarray(inputs["t"], np.float32)
    A = (np.asarray(inputs["Wq"], np.float32)
         @ np.asarray(inputs["Wk"], np.float32).T).astype(bf)
    Wv = np.asarray(inputs["Wv"], np.float32)
    # per-core column order: [peer's channel half | own half]
    Wv_p = [np.ascontiguousarray(
        np.concatenate([Wv[:, (1 - g) * CH:(2 - g) * CH],
                        Wv[:, g * CH:(g + 1) * CH]], axis=1)).astype(bf)
        for g in range(2)]
    Wp = np.asarray(inputs["Wp"], np.float32).astype(bf)
    bp = np.asarray(inputs["bp"], np.float32).reshape(1, C)
    xTs = [np.ascontiguousarray(x[b].T).astype(bf) for b in range(B)]
    sels = [np.concatenate([np.full(512, float(g == 0), np.float32),
                            np.full(512, float(g == 1), np.float32)]
                           ).reshape(1, 1024)
            for g in range(2)]
    maps = []
    for p in range(N_CORES):
        b, g = p // 2, p % 2
        maps.append({
            "xT": xTs[b],
            "tT": np.ascontiguousarray(np.concatenate(
                ([t[b, g * QS:(g + 1) * QS - QR],
                  t[b, QS - QR:QS], t[b, 2 * QS - QR:2 * QS]]
                 if TAIL_LOCAL else
                 [t[b, g * QS:(g + 1) * QS],
                  np.zeros((QR, C), np.float32)]),
                axis=0).T).astype(bf),
            "A": A, "Wv": Wv_p[g], "Wp": Wp, "bp": bp, "sel": sels[g],
        })
    return maps


def assemble(results):
    out = np.empty((B, TN, C), np.float32)
    for p in range(N_CORES):
        b, g = p // 2, p % 2
        out[b, g * QS:(g + 1) * QS] = results[p]["out"]
    return out


_NC_CACHE = {}


def _get_nc(repeat=1):
    if repeat not in _NC_CACHE:
        _NC_CACHE[repeat] = build(repeat=repeat)
    return _NC_CACHE[repeat]


def kernel(**inputs) -> np.ndarray:
    nc = _get_nc()
    in_maps = make_in_maps(inputs)
    res = run_bass_kernel_spmd(nc, in_maps, list(range(N_CORES)))
    return assemble(res.results)


# revision 35
# speedup vs baseline: 1.1277x; 1.1277x over previous
"""nn_CrossAttention TRN2 kernel v4 — 8-core SPMD Bass/Tile, query-sharded.

Sharding: core p -> batch b = p//2, query-half g = p%2.
Each core: its 2048 queries against all 4096 keys -> softmax completes
locally (no partial-D exchange).

Host-side prep (layout/dtype only + weight folding):
  xT, tT   channel-major bf16 transposes of x / t (kills all input
           PE-transposes and casts; halves input DMA bytes)
  A        Wq @ Wk.T (f32 fold, bf16 cast) -> scores = (t A) x^T, so the
           k-projection disappears and xT is the score stationary operand

Per-core dataflow:
  qAT      = A^T tT   channel-major bf16 [384, 2048]
  v        = x Wv'    row-major bf16 [keys, 384]; Wv' is Wv with columns
           permuted per-core to [peer's channel half | own half] so the
           exchange slicing is rank-INDEPENDENT (SPMD single program)
  s->e->o  per chunk (512 queries) x n (128 keys): scores (3 MM), exp
           (ACT), o^T accumulation (3 MM), software-pipelined ACROSS chunk
           boundaries so the PE stream never drains; the softmax
           denominator accumulates on DVE (no PE ones-matmul) and is
           partition-reduced once per chunk on GpSimd
  normalize per chunk: reciprocal -> partition-broadcast -> 3 DVE muls
           (PSUM -> bf16 SBUF), all-f32 denominators
  repeat pipelining: the next pass's input DMAs + qA/v projections are
           emitted between a pass's attention and its tail, so the PE
           fills the final-exchange latency with useful work and HAM
           stays warm into the projection
  AllGather per T: pairwise, ships only the peer-destined 192 channels
           (half the ReduceScatter wire bytes, pure data movement, no
           partial-D exchange). The received block and the locally-kept
           half are placed into the channel-major zbuf via exact 0/1
           mask multiplies (masks are host inputs = the core's rank bit),
           keeping every instruction rank-independent.
  zbuf     channel-major [192, 4096]: the "transpose(1,2).reshape"
           permutation becomes contiguous rows
  out proj reads zbuf rows, PE-transposes, Wp matmul; bias via DVE add
"""
from contextlib import ExitStack

import numpy as np
import ml_dtypes

import concourse.bass as bass
import concourse.bass_isa as bass_isa
import concourse.tile as tile
from concourse import bacc, mybir
from concourse.bass_utils import run_bass_kernel_spmd
from concourse.masks import make_identity

F32 = mybir.dt.float32
BF16 = mybir.dt.bfloat16
EXP = mybir.ActivationFunctionType.Exp

B, N, TN, C = 4, 4096, 4096, 384
QS = TN // 2           # queries per core
CH = C // 2            # o^T channels per core after exchange
SCALE = (C // 8) ** -0.5
N_CORES = 8
GROUPS = [[0, 1], [2, 3], [4, 5], [6, 7]]
TAIL_LOCAL = False     # compute the last QR queries of BOTH ranks locally
QR = 256               # redundantly-computed peer queries (tail chunk)
QE = QS + QR           # query columns per core incl. peer tail
# (start, width) of attention chunks over the QE query columns; the last
# chunk = own tail QR + peer tail QR, computed locally on BOTH cores so no
# collective sits on the critical tail, and the preceding exchanges hide
# under the later chunks' compute
CHUNKS = [(0, 512), (512, 512), (1024, 512), (1536, QS - 1536 - QR),
          (QS - QR, 2 * QR)]


def build(repeat=1, stop_after=None, sim_cc=False, tail_local=TAIL_LOCAL):
    nc = bacc.Bacc("TRN2", target_bir_lowering=False, debug=False,
                   num_devices=N_CORES)
    nc._sim_cc = sim_cc
    nc._tail_local = tail_local
    xT_d = nc.dram_tensor("xT", [C, N], BF16, kind="ExternalInput").ap()
    tT_d = nc.dram_tensor("tT", [C, QE], BF16, kind="ExternalInput").ap()
    w_d = {n: nc.dram_tensor(n, [C, C], BF16, kind="ExternalInput").ap()
           for n in ("A", "Wv", "Wp")}
    bp_d = nc.dram_tensor("bp", [1, C], F32, kind="ExternalInput").ap()
    sel_d = nc.dram_tensor("sel", [1, 1024], F32, kind="ExternalInput").ap()
    out_d = nc.dram_tensor("out", [QS, C], F32, kind="ExternalOutput").ap()

    with tile.TileContext(nc) as tc:
        _kernel_body(nc, tc, xT_d, tT_d, w_d, bp_d, sel_d, out_d, repeat,
                     stop_after)
    nc.compile()
    return nc


def _kernel_body(nc, tc, xT_d, tT_d, w_d, bp_d, sel_d, out_d, repeat,
                 stop_after):
    with ExitStack() as ctx:
        consts = ctx.enter_context(tc.tile_pool(name="consts", bufs=1))
        dram = ctx.enter_context(tc.tile_pool(name="dram", bufs=1, space="DRAM"))

        ident = consts.tile([128, 128], BF16)
        make_identity(nc, ident)

        # weights / bias: loaded once (not in the repeat loop)
        w_sb = {}
        for name in ("A", "Wv", "Wp"):
            cw = consts.tile([128, 3 * C], BF16, name=f"{name}_sb",
                             tag=f"{name}_sb")
            for dc in range(3):
                nc.sync.dma_start(cw[:, dc * C:(dc + 1) * C],
                                  w_d[name][dc * 128:(dc + 1) * 128, :])
            w_sb[name] = cw
        bp_sb = consts.tile([1, C], F32, name="bp_sb", tag="bp_sb")
        nc.sync.dma_start(bp_sb[:], bp_d[:])
        bias_bc = consts.tile([128, C], F32, name="bias_bc", tag="bias_bc")
        nc.gpsimd.partition_broadcast(bias_bc[:], bp_sb[:])

        # rank-bit masks: sel row0 = (own rank is group-rank 0) as 0/1,
        # row1 = the complement.  Broadcast to full tiles once.
        sel_sb = consts.tile([1, 1024], F32, name="sel_sb", tag="sel_sb")
        nc.sync.dma_start(sel_sb[:], sel_d[:])
        s_bc = consts.tile([128, 512], F32, name="s_bc", tag="s_bc")
        nc.gpsimd.partition_broadcast(s_bc[:], sel_sb[:, 0:512])
        m_bc = consts.tile([128, 512], F32, name="m_bc", tag="m_bc")
        nc.gpsimd.partition_broadcast(m_bc[:], sel_sb[:, 512:1024])

        def wch(name, dc, cc=None):
            if cc is None:
                return w_sb[name][:, dc * C:(dc + 1) * C]
            return w_sb[name][:, dc * C + cc * 128: dc * C + (cc + 1) * 128]

        chunks = CHUNKS if getattr(nc, "_tail_local", True) else \
            [(0, 512), (512, 512), (1024, 512), (1536, 512)]
        args = (nc, tc, xT_d, tT_d, out_d, ident, wch, bias_bc, s_bc, m_bc,
                dram, chunks)
        st = _stage_load(*args)
        for rep in range(repeat):
            mid = _stage_attn(st, rep, stop_after, *args)
            if stop_after == "load" or stop_after == "att2":
                if rep + 1 < repeat:
                    st = _stage_load(*args)
                continue
            # next rep's loads + qA + v emit BEFORE this rep's tail, so the
            # PE fills the final-exchange gap with useful work (and HAM
            # stays warm into the projection)
            if rep + 1 < repeat:
                st = _stage_load(*args)
            _stage_tail(st if rep + 1 >= repeat else None, mid, rep,
                        *args)


def _stage_load(nc, tc, xT_d, tT_d, out_d, ident, wch, bias_bc, s_bc,
                m_bc, dram, chunks):
    NK = N // 128
    attin = tc.alloc_tile_pool(name="attin", bufs=1)
    # ---- input loads: tT first (unblocks qA), column-chunked ----
    tT = [attin.tile([128, QE], BF16, name=f"tT{dc}", tag=f"tT{dc}")
          for dc in range(3)]
    for q0, w in chunks:
        for dc in range(3):
            nc.sync.dma_start(tT[dc][:, q0:q0 + w],
                              tT_d[dc * 128:(dc + 1) * 128, q0:q0 + w])
    xT = [attin.tile([128, N], BF16, name=f"xT{cc}", tag=f"xT{cc}")
          for cc in range(3)]
    for nx in range(4):
        xsl = slice(nx * (N // 4), (nx + 1) * (N // 4))
        for cc in range(3):
            nc.sync.dma_start(xT[cc][:, xsl],
                              xT_d[cc * 128:(cc + 1) * 128, xsl])

    # ---- qAT = A^T tT ----
    qAT = [attin.tile([128, QE], BF16, name=f"qAT{cc}", tag=f"qAT{cc}")
           for cc in range(3)]
    with tc.tile_pool(name="qpsum", bufs=2, space="PSUM") as qpsum:
        for q0, w in chunks:
            for cc in range(3):
                ps = qpsum.tile([128, 512], F32, name="qps", tag="qps")
                for dc in range(3):
                    nc.tensor.matmul(
                        ps[:, 0:w], wch("A", dc, cc),
                        tT[dc][:, q0:q0 + w],
                        start=(dc == 0), stop=(dc == 2))
                nc.scalar.copy(qAT[cc][:, q0:q0 + w], ps[:, 0:w])

    # ---- v = x Wv ----
    v_all = attin.tile([128, NK * C], BF16, name="v_all", tag="v_all")
    with tc.tile_pool(name="vpsum", bufs=3, space="PSUM") as vpsum:
        for n in range(NK):
            ps = vpsum.tile([128, C], F32, name="vps", tag="vps")
            for dc in range(3):
                nc.tensor.matmul(
                    ps[:], xT[dc][:, n * 128:(n + 1) * 128],
                    wch("Wv", dc), start=(dc == 0), stop=(dc == 2))
            nc.vector.tensor_copy(v_all[:, n * C:(n + 1) * C], ps[:])
    return (attin, xT, tT, qAT, v_all)


def _stage_attn(st, rep, stop_after, nc, tc, xT_d, tT_d, out_d, ident, wch,
                bias_bc, s_bc, m_bc, dram, chunks):
    NK = N // 128
    attin, xT, tT, qAT, v_all = st
    if stop_after == "load":
        attin.release()
        return None
    zbuf = dram.tile([QS, C], BF16, name=f"zbuf{rep}", tag="zbuf")
    zchan = zbuf[:].rearrange("a b -> (a b)").rearrange("(c t) -> c t",
                                                        t=N)
    sbufs = [dram.tile([CH, w], BF16, name=f"sb{rep}_{T}", tag=f"sb{T}")
             for T, (q0, w) in enumerate(chunks) if _ships(nc, T, chunks)]
    rbufs = [dram.tile([2, CH, w], BF16, name=f"rb{rep}_{T}", tag=f"rb{T}")
             for T, (q0, w) in enumerate(chunks) if _ships(nc, T, chunks)]

    npool = tc.alloc_tile_pool(name="npool", bufs=2, side="right")
    with tc.tile_pool(name="spsum", bufs=2, space="PSUM") as spsum, \
         tc.tile_pool(name="opsum", bufs=2, space="PSUM") as opsum, \
         tc.tile_pool(name="epool", bufs=8, side="right") as epool:
        prev = None

        def flush_prev():
            nonlocal prev
            if prev is not None:
                e_p, n_p, ops_p, w_p = prev
                for cc in range(3):
                    nc.tensor.matmul(
                        ops_p[cc][:, 0:w_p],
                        v_all[:, n_p * C + cc * 128:
                              n_p * C + (cc + 1) * 128],
                        e_p[:, 0:w_p],
                        start=(n_p == 0), stop=(n_p == NK - 1))
                prev = None

        o_sets, e_sums = [], []
        for T, (q0, w) in enumerate(chunks):
            o_ps = [opsum.tile([128, 512], F32, name=f"ops{cc}",
                               tag=f"ops{cc}") for cc in range(3)]
            e_sum = epool.tile([128, 512], F32, name="e_sum", tag="e_sum")
            o_sets.append(o_ps)
            e_sums.append(e_sum)
            for n in range(NK):
                s_ps = spsum.tile([128, 512], F32, name="sps", tag="sps")
                for cc in range(3):
                    nc.tensor.matmul(
                        s_ps[:, 0:w], xT[cc][:, n * 128:(n + 1) * 128],
                        qAT[cc][:, q0:q0 + w],
                        start=(cc == 0), stop=(cc == 2))
                e_t = epool.tile([128, 512], BF16, name="e_t", tag="e_t")
                nc.scalar.activation(e_t[:, 0:w], s_ps[:, 0:w], EXP,
                                     scale=SCALE)
                # denominator accumulates on DVE (PE freed of the
                # ones-matmul); partition-reduced once per chunk
                if n == 0:
                    nc.vector.tensor_copy(e_sum[:, 0:w], e_t[:, 0:w])
                else:
                    nc.vector.tensor_add(e_sum[:, 0:w], e_sum[:, 0:w],
                                         e_t[:, 0:w])
                flush_prev()
                prev = (e_t, n, o_ps, w)
                # the n==0 flush above issued the previous chunk's last
                # o-matmuls -> ship it now so the exchange overlaps this
                # chunk's compute
                if n == 0 and T > 0:
                    oTn_p = _norm(nc, T - 1, o_sets[T - 1], e_sums[T - 1],
                                  npool, chunks)
                    _ship(nc, T - 1, oTn_p, npool, sbufs, rbufs, zchan,
                          s_bc, m_bc, chunks)
        flush_prev()
        # the last chunk's normalize needs the attention PSUM; emit it here
        # so spsum/opsum/epool can release before the next pass's
        # projections -- only the ship remains for the tail stage
        oTn_last = _norm(nc, len(chunks) - 1, o_sets[-1], e_sums[-1],
                         npool, chunks)
    attin.release()
    return (npool, oTn_last, sbufs, rbufs, zbuf, zchan)


def _stage_tail(next_st, mid, rep, nc, tc, xT_d, tT_d, out_d, ident, wch,
                bias_bc, s_bc, m_bc, dram, chunks):
    if mid is None:
        return
    npool, oTn_last, sbufs, rbufs, zbuf, zchan = mid
    _ship(nc, len(chunks) - 1, oTn_last, npool, sbufs, rbufs, zchan,
          s_bc, m_bc, chunks)
    npool.release()

    # ---- output projection over own 2048 rows ----
    NF = QS // 128
    with tc.tile_pool(name="fpool", bufs=4) as fpool, \
         tc.tile_pool(name="fpsum", bufs=4, space="PSUM") as fpsum, \
         tc.tile_pool(name="ftpsum", bufs=4, space="PSUM") as ftpsum:
        def f_load(it):
            r_t = fpool.tile([128, C], BF16, name="r_t", tag="r_t")
            nc.sync.dma_start(r_t[:], zbuf[it * 128:(it + 1) * 128, :])
            bank = ftpsum.tile([128, C], BF16, name="f_tr", tag="f_tr")
            for jc in range(3):
                nc.tensor.transpose(bank[:, jc * 128:(jc + 1) * 128],
                                    r_t[:, jc * 128:(jc + 1) * 128], ident[:])
            op_ch = fpool.tile([128, C], BF16, name="op_ch", tag="op_ch")
            nc.scalar.copy(op_ch[:], bank[:])
            return op_ch

        op_q = [f_load(0), f_load(1)]
        for it in range(NF):
            op_ch = op_q.pop(0)
            if it + 2 < NF:
                op_q.append(f_load(it + 2))
            out_ps = fpsum.tile([128, C], F32, name="out_ps", tag="out_ps")
            for jc in range(3):
                nc.tensor.matmul(out_ps[:], op_ch[:, jc * 128:(jc + 1) * 128],
                                 wch("Wp", jc), start=(jc == 0),
                                 stop=(jc == 2))
            o_t = fpool.tile([128, C], F32, name="o_t", tag="o_t")
            nc.vector.tensor_add(o_t[:], out_ps[:], bias_bc[:])
            nc.sync.dma_start(out_d[it * 128:(it + 1) * 128, :], o_t[:])


def _ships(nc, T, chunks):
    return not (getattr(nc, "_tail_local", True) and T == len(chunks) - 1)


def _norm(nc, T, o_ps, e_sum, npool, chunks):
    q0, w = chunks[T]
    e_red = npool.tile([128, 512], F32, name="e_red", tag="e_red")
    nc.gpsimd.partition_all_reduce(e_red[:, 0:w], e_sum[:, 0:w], 128,
                                   bass_isa.ReduceOp.add)
    rec = npool.tile([1, 512], F32, name="rec", tag="rec")
    nc.vector.reciprocal(rec[:, 0:w], e_red[0:1, 0:w])
    rec_b = npool.tile([128, 512], F32, name="rec_b", tag="rec_b")
    nc.gpsimd.partition_broadcast(rec_b[:, 0:w], rec[:, 0:w])
    oTn = [npool.tile([128, 512], BF16, name=f"oTn{cc}", tag=f"oTn{cc}")
           for cc in range(3)]
    for cc in range(3):
        nc.vector.tensor_mul(oTn[cc][:, 0:w], o_ps[cc][:, 0:w],
                             rec_b[:, 0:w])
    return oTn


def _ship(nc, T, oTn, npool, sbufs, rbufs, zchan, s_bc, m_bc, chunks):
    q0, w = chunks[T]
    last = not _ships(nc, T, chunks)
    coef = (s_bc, m_bc)
    if not last:
        # ship peer-destined half (oTn rows 0..192 by Wv-perm)
        sb = sbufs[T]
        nc.sync.dma_start(sb[0:128, :], oTn[0][:, 0:w])
        nc.sync.dma_start(sb[128:CH, :], oTn[1][0:64, 0:w])
        if getattr(nc, "_sim_cc", False):
            for j in range(2):
                nc.sync.dma_start(rbufs[T][j], sb[:])
        else:
            nc.gpsimd.collective_compute(
                "AllGather", mybir.AluOpType.bypass,
                replica_groups=GROUPS,
                ins=[sb[:].opt()], outs=[rbufs[T][:].opt()])
        # place own half + received peer half into zchan at their GLOBAL
        # query columns, selected by exact 0/1 rank masks:
        #   col-block j (global cols j*QS+q0) =
        #       own * (j == my rank) + rb[j] * (j == peer rank)
        rbt = [npool.tile([128, 512], BF16, name=f"rba{j}", tag=f"rba{j}")
               for j in range(2)]
        rbb = [npool.tile([128, 512], BF16, name=f"rbb{j}", tag=f"rbb{j}")
               for j in range(2)]
        for j in range(2):
            # top 64 received rows land on partitions 64..128 so every DVE
            # op below is partition-aligned with the own rows of oTn[1]
            nc.sync.dma_start(rbt[j][64:128, 0:w], rbufs[T][j, 0:64, :])
            nc.sync.dma_start(rbb[j][:, 0:w], rbufs[T][j, 64:CH, :])
        own_a = oTn[1][64:128, 0:w]
        own_b = oTn[2][:, 0:w]
        for j in range(2):
            za = npool.tile([128, 512], BF16, name=f"za{j}", tag=f"za{j}")
            zb = npool.tile([128, 512], BF16, name=f"zb{j}", tag=f"zb{j}")
            ta = npool.tile([128, 512], BF16, name=f"ta{j}", tag=f"ta{j}")
            tb = npool.tile([128, 512], BF16, name=f"tb{j}", tag=f"tb{j}")
            nc.vector.tensor_mul(za[64:128, 0:w], own_a,
                                 coef[j][64:128, 0:w])
            nc.vector.tensor_mul(zb[:, 0:w], own_b, coef[j][:, 0:w])
            nc.vector.tensor_mul(ta[64:128, 0:w], rbt[j][64:128, 0:w],
                                 coef[1 - j][64:128, 0:w])
            nc.vector.tensor_mul(tb[:, 0:w], rbb[j][:, 0:w],
                                 coef[1 - j][:, 0:w])
            nc.vector.tensor_add(za[64:128, 0:w], za[64:128, 0:w],
                                 ta[64:128, 0:w])
            nc.vector.tensor_add(zb[:, 0:w], zb[:, 0:w], tb[:, 0:w])
            csl = slice(j * QS + q0, j * QS + q0 + w)
            nc.sync.dma_start(zchan[0:64, csl], za[64:128, 0:w])
            nc.sync.dma_start(zchan[64:CH, csl], zb[:, 0:w])
    else:
        # tail chunk, fully local on both cores and packed in RANK order by
        # the host (rank 0's tail queries then rank 1's, identical on both
        # cores), so the z writes are fixed slices -- no masks at all
        for j in range(2):
            csl = slice(j * QS + q0, j * QS + q0 + QR)
            nc.sync.dma_start(zchan[0:64, csl],
                              oTn[1][64:128, j * QR:(j + 1) * QR])
            nc.sync.dma_start(zchan[64:CH, csl],
                              oTn[2][:, j * QR:(j + 1) * QR])


def make_in_maps(inputs):
    bf = ml_dtypes.bfloat16
    x = np.asarray(inputs["x"], np.float32)
    t = np.asarray(inputs["t"], np.float32)
    A = (np.asarray(inputs["Wq"], np.float32)
         @ np.asarray(inputs["Wk"], np.float32).T).astype(bf)
    Wv = np.asarray(inputs["Wv"], np.float32)
    # per-core column order: [peer's channel half | own half]
    Wv_p = [np.ascontiguousarray(
        np.concatenate([Wv[:, (1 - g) * CH:(2 - g) * CH],
                        Wv[:, g * CH:(g + 1) * CH]], axis=1)).astype(bf)
        for g in range(2)]
    Wp = np.asarray(inputs["Wp"], np.float32).astype(bf)
    bp = np.asarray(inputs["bp"], np.float32).reshape(1, C)
    xTs = [np.ascontiguousarray(x[b].T).astype(bf) for b in range(B)]
    sels = [np.concatenate([np.full(512, float(g == 0), np.float32),
                            np.full(512, float(g == 1), np.float32)]
                           ).reshape(1, 1024)
            for g in range(2)]
    maps = []
    for p in range(N_CORES):
        b, g = p // 2, p % 2
        maps.append({
            "xT": xTs[b],
            "tT": np.ascontiguousarray(np.concatenate(
                ([t[b, g * QS:(g + 1) * QS - QR],
                  t[b, QS - QR:QS], t[b, 2 * QS - QR:2 * QS]]
                 if TAIL_LOCAL else
                 [t[b, g * QS:(g + 1) * QS],
                  np.zeros((QR, C), np.float32)]),
                axis=0).T).astype(bf),
            "A": A, "Wv": Wv_p[g], "Wp": Wp, "bp": bp, "sel": sels[g],
        })
    return maps


def assemble(results):
    out = np.empty((B, TN, C), np.float32)
    for p in range(N_CORES):
        b, g = p // 2, p % 2
        out[b, g * QS:(g + 1) * QS] = results[p]["out"]
    return out


_NC_CACHE = {}


def _get_nc(repeat=1):
    if repeat not in _NC_CACHE:
        _NC_CACHE[repeat] = build(repeat=repeat)
    return _NC_CACHE[repeat]


def kernel(**inputs) -> np.ndarray:
    nc = _get_nc()
    in_maps = make_in_maps(inputs)
    res = run_bass_kernel_spmd(nc, in_maps, list(range(N_CORES)))
    return assemble(res.results)


# revision 36
# speedup vs baseline: 1.2182x; 1.0803x over previous
"""nn_CrossAttention TRN2 kernel v4 — 8-core SPMD Bass/Tile, query-sharded.

Sharding: core p -> batch b = p//2, query-half g = p%2.
Each core: its 2048 queries against all 4096 keys -> softmax completes
locally (no partial-D exchange).

Host-side prep (layout/dtype only + weight folding):
  xT, tT   channel-major bf16 transposes of x / t (kills all input
           PE-transposes and casts; halves input DMA bytes)
  A        Wq @ Wk.T (f32 fold, bf16 cast) -> scores = (t A) x^T, so the
           k-projection disappears and xT is the score stationary operand

Per-core dataflow:
  qAT      = A^T tT   channel-major bf16 [384, 2048]
  v        = x Wv'    row-major bf16 [keys, 384]; Wv' is Wv with columns
           permuted per-core to [peer's channel half | own half] so the
           exchange slicing is rank-INDEPENDENT (SPMD single program)
  s->e->o  per chunk (512 queries) x n (128 keys): scores (3 MM), exp
           (ACT), o^T accumulation (3 MM), software-pipelined ACROSS chunk
           boundaries so the PE stream never drains; the softmax
           denominator accumulates on DVE (no PE ones-matmul) and is
           partition-reduced once per chunk on GpSimd
  normalize per chunk: reciprocal -> partition-broadcast -> 3 DVE muls
           (PSUM -> bf16 SBUF), all-f32 denominators
  repeat pipelining: the next pass's input DMAs + qA/v projections are
           emitted between a pass's attention and its tail, so the PE
           fills the final-exchange latency with useful work and HAM
           stays warm into the projection
  AllGather per T: pairwise, ships only the peer-destined 192 channels
           (half the ReduceScatter wire bytes, pure data movement, no
           partial-D exchange). The received block and the locally-kept
           half are placed into the channel-major zbuf via exact 0/1
           mask multiplies (masks are host inputs = the core's rank bit),
           keeping every instruction rank-independent.
  zbuf     channel-major [192, 4096]: the "transpose(1,2).reshape"
           permutation becomes contiguous rows
  out proj reads zbuf rows, PE-transposes, Wp matmul; bias via DVE add
"""
from contextlib import ExitStack

import numpy as np
import ml_dtypes

import concourse.bass as bass
import concourse.bass_isa as bass_isa
import concourse.tile as tile
from concourse import bacc, mybir
from concourse.bass_utils import run_bass_kernel_spmd
from concourse.masks import make_identity

F32 = mybir.dt.float32
BF16 = mybir.dt.bfloat16
EXP = mybir.ActivationFunctionType.Exp

B, N, TN, C = 4, 4096, 4096, 384
QS = TN // 2           # queries per core
CH = C // 2            # o^T channels per core after exchange
SCALE = (C // 8) ** -0.5
N_CORES = 8
GROUPS = [[0, 1], [2, 3], [4, 5], [6, 7]]
TAIL_LOCAL = False     # compute the last QR queries of BOTH ranks locally
QR = 256               # redundantly-computed peer queries (tail chunk)
QE = QS + QR           # query columns per core incl. peer tail
# (start, width) of attention chunks over the QE query columns; the last
# chunk = own tail QR + peer tail QR, computed locally on BOTH cores so no
# collective sits on the critical tail, and the preceding exchanges hide
# under the later chunks' compute
CHUNKS = [(0, 512), (512, 512), (1024, 512), (1536, QS - 1536 - QR),
          (QS - QR, 2 * QR)]


def build(repeat=1, stop_after=None, sim_cc=False, tail_local=TAIL_LOCAL):
    nc = bacc.Bacc("TRN2", target_bir_lowering=False, debug=False,
                   num_devices=N_CORES)
    nc._sim_cc = sim_cc
    nc._tail_local = tail_local
    xT_d = nc.dram_tensor("xT", [C, N], BF16, kind="ExternalInput").ap()
    tT_d = nc.dram_tensor("tT", [C, QE], BF16, kind="ExternalInput").ap()
    w_d = {n: nc.dram_tensor(n, [C, C], BF16, kind="ExternalInput").ap()
           for n in ("A", "Wv", "Wp")}
    bp_d = nc.dram_tensor("bp", [1, C], F32, kind="ExternalInput").ap()
    sel_d = nc.dram_tensor("sel", [1, 1024], F32, kind="ExternalInput").ap()
    out_d = nc.dram_tensor("out", [QS, C], F32, kind="ExternalOutput").ap()

    with tile.TileContext(nc) as tc:
        _kernel_body(nc, tc, xT_d, tT_d, w_d, bp_d, sel_d, out_d, repeat,
                     stop_after)
    nc.compile()
    return nc


def _kernel_body(nc, tc, xT_d, tT_d, w_d, bp_d, sel_d, out_d, repeat,
                 stop_after):
    with ExitStack() as ctx:
        consts = ctx.enter_context(tc.tile_pool(name="consts", bufs=1))
        dram = ctx.enter_context(tc.tile_pool(name="dram", bufs=1, space="DRAM"))

        ident = consts.tile([128, 128], BF16)
        make_identity(nc, ident)

        # weights / bias: loaded once (not in the repeat loop)
        w_sb = {}
        for name in ("A", "Wv", "Wp"):
            cw = consts.tile([128, 3 * C], BF16, name=f"{name}_sb",
                             tag=f"{name}_sb")
            for dc in range(3):
                nc.sync.dma_start(cw[:, dc * C:(dc + 1) * C],
                                  w_d[name][dc * 128:(dc + 1) * 128, :])
            w_sb[name] = cw
        bp_sb = consts.tile([1, C], F32, name="bp_sb", tag="bp_sb")
        nc.sync.dma_start(bp_sb[:], bp_d[:])
        bias_bc = consts.tile([128, C], F32, name="bias_bc", tag="bias_bc")
        nc.gpsimd.partition_broadcast(bias_bc[:], bp_sb[:])

        # rank-bit masks: sel row0 = (own rank is group-rank 0) as 0/1,
        # row1 = the complement.  Broadcast to full tiles once.
        sel_sb = consts.tile([1, 1024], F32, name="sel_sb", tag="sel_sb")
        nc.sync.dma_start(sel_sb[:], sel_d[:])
        s_bc = consts.tile([128, 512], F32, name="s_bc", tag="s_bc")
        nc.gpsimd.partition_broadcast(s_bc[:], sel_sb[:, 0:512])
        m_bc = consts.tile([128, 512], F32, name="m_bc", tag="m_bc")
        nc.gpsimd.partition_broadcast(m_bc[:], sel_sb[:, 512:1024])

        def wch(name, dc, cc=None):
            if cc is None:
                return w_sb[name][:, dc * C:(dc + 1) * C]
            return w_sb[name][:, dc * C + cc * 128: dc * C + (cc + 1) * 128]

        chunks = CHUNKS if getattr(nc, "_tail_local", True) else \
            [(0, 512), (512, 512), (1024, 512), (1536, 512)]
        args = (nc, tc, xT_d, tT_d, out_d, ident, wch, bias_bc, s_bc, m_bc,
                dram, chunks)
        st = _stage_load(*args)
        for rep in range(repeat):
            mid = _stage_attn(st, rep, stop_after, *args)
            if stop_after == "load" or stop_after == "att2":
                if rep + 1 < repeat:
                    st = _stage_load(*args)
                continue
            # next rep's loads + qA + v emit BEFORE this rep's tail, so the
            # PE fills the final-exchange gap with useful work (and HAM
            # stays warm into the projection)
            if rep + 1 < repeat:
                st = _stage_load(*args)
            _stage_tail(st if rep + 1 >= repeat else None, mid, rep,
                        *args)


def _stage_load(nc, tc, xT_d, tT_d, out_d, ident, wch, bias_bc, s_bc,
                m_bc, dram, chunks):
    NK = N // 128
    attin = tc.alloc_tile_pool(name="attin", bufs=1)
    # ---- input loads: tT first (unblocks qA), column-chunked ----
    tT = [attin.tile([128, QE], BF16, name=f"tT{dc}", tag=f"tT{dc}")
          for dc in range(3)]
    for q0, w in chunks:
        for dc in range(3):
            nc.sync.dma_start(tT[dc][:, q0:q0 + w],
                              tT_d[dc * 128:(dc + 1) * 128, q0:q0 + w])
    xT = [attin.tile([128, N], BF16, name=f"xT{cc}", tag=f"xT{cc}")
          for cc in range(3)]
    for nx in range(4):
        xsl = slice(nx * (N // 4), (nx + 1) * (N // 4))
        for cc in range(3):
            nc.sync.dma_start(xT[cc][:, xsl],
                              xT_d[cc * 128:(cc + 1) * 128, xsl])

    # ---- qAT = A^T tT ----
    qAT = [attin.tile([128, QE], BF16, name=f"qAT{cc}", tag=f"qAT{cc}")
           for cc in range(3)]
    with tc.tile_pool(name="qpsum", bufs=2, space="PSUM") as qpsum:
        for q0, w in chunks:
            for cc in range(3):
                ps = qpsum.tile([128, 512], F32, name="qps", tag="qps")
                for dc in range(3):
                    nc.tensor.matmul(
                        ps[:, 0:w], wch("A", dc, cc),
                        tT[dc][:, q0:q0 + w],
                        start=(dc == 0), stop=(dc == 2))
                nc.scalar.copy(qAT[cc][:, q0:q0 + w], ps[:, 0:w])

    # ---- v = x Wv ----
    v_all = attin.tile([128, NK * C], BF16, name="v_all", tag="v_all")
    with tc.tile_pool(name="vpsum", bufs=3, space="PSUM") as vpsum:
        for n in range(NK):
            ps = vpsum.tile([128, C], F32, name="vps", tag="vps")
            for dc in range(3):
                nc.tensor.matmul(
                    ps[:], xT[dc][:, n * 128:(n + 1) * 128],
                    wch("Wv", dc), start=(dc == 0), stop=(dc == 2))
            nc.vector.tensor_copy(v_all[:, n * C:(n + 1) * C], ps[:])
    return (attin, xT, tT, qAT, v_all)


def _stage_attn(st, rep, stop_after, nc, tc, xT_d, tT_d, out_d, ident, wch,
                bias_bc, s_bc, m_bc, dram, chunks):
    NK = N // 128
    attin, xT, tT, qAT, v_all = st
    if stop_after == "load":
        attin.release()
        return None
    zbuf = dram.tile([QS, C], BF16, name=f"zbuf{rep}", tag="zbuf")
    zchan = zbuf[:].rearrange("a b -> (a b)").rearrange("(c t) -> c t",
                                                        t=N)
    sbufs = [dram.tile([CH, w], BF16, name=f"sb{rep}_{T}", tag=f"sb{T}")
             for T, (q0, w) in enumerate(chunks) if _ships(nc, T, chunks)]
    rbufs = [dram.tile([2, CH, w], BF16, name=f"rb{rep}_{T}", tag=f"rb{T}")
             for T, (q0, w) in enumerate(chunks) if _ships(nc, T, chunks)]

    npool = tc.alloc_tile_pool(name="npool", bufs=2, side="right")
    with tc.tile_pool(name="spsum", bufs=2, space="PSUM") as spsum, \
         tc.tile_pool(name="opsum", bufs=2, space="PSUM") as opsum, \
         tc.tile_pool(name="epool", bufs=8, side="right") as epool:
        prev = None

        def flush_prev():
            nonlocal prev
            if prev is not None:
                e_p, n_p, ops_p, w_p = prev
                for cc in range(3):
                    nc.tensor.matmul(
                        ops_p[cc][:, 0:w_p],
                        v_all[:, n_p * C + cc * 128:
                              n_p * C + (cc + 1) * 128],
                        e_p[:, 0:w_p],
                        start=(n_p == 0), stop=(n_p == NK - 1))
                prev = None

        o_sets, e_sums = [], []
        for T, (q0, w) in enumerate(chunks):
            o_ps = [opsum.tile([128, 512], F32, name=f"ops{cc}",
                               tag=f"ops{cc}") for cc in range(3)]
            e_sum = epool.tile([128, 512], F32, name="e_sum", tag="e_sum")
            o_sets.append(o_ps)
            e_sums.append(e_sum)
            for n in range(NK):
                s_ps = spsum.tile([128, 512], F32, name="sps", tag="sps")
                for cc in range(3):
                    nc.tensor.matmul(
                        s_ps[:, 0:w], xT[cc][:, n * 128:(n + 1) * 128],
                        qAT[cc][:, q0:q0 + w],
                        start=(cc == 0), stop=(cc == 2))
                e_t = epool.tile([128, 512], BF16, name="e_t", tag="e_t")
                nc.scalar.activation(e_t[:, 0:w], s_ps[:, 0:w], EXP,
                                     scale=SCALE)
                # denominator accumulates on DVE (PE freed of the
                # ones-matmul); partition-reduced once per chunk
                if n == 0:
                    nc.vector.tensor_copy(e_sum[:, 0:w], e_t[:, 0:w])
                else:
                    nc.vector.tensor_add(e_sum[:, 0:w], e_sum[:, 0:w],
                                         e_t[:, 0:w])
                flush_prev()
                prev = (e_t, n, o_ps, w)
                # the n==0 flush above issued the previous chunk's last
                # o-matmuls -> ship it now so the exchange overlaps this
                # chunk's compute
                if n == 0 and T > 0:
                    oTn_p = _norm(nc, T - 1, o_sets[T - 1], e_sums[T - 1],
                                  npool, chunks)
                    _ship(nc, T - 1, oTn_p, npool, sbufs, rbufs, zchan,
                          s_bc, m_bc, chunks)
        flush_prev()
        # the last chunk's normalize needs the attention PSUM; emit it here
        # so spsum/opsum/epool can release before the next pass's
        # projections -- only the ship remains for the tail stage
        oTn_last = _norm(nc, len(chunks) - 1, o_sets[-1], e_sums[-1],
                         npool, chunks)
    attin.release()
    return (npool, oTn_last, sbufs, rbufs, zbuf, zchan)


def _stage_tail(next_st, mid, rep, nc, tc, xT_d, tT_d, out_d, ident, wch,
                bias_bc, s_bc, m_bc, dram, chunks):
    if mid is None:
        return
    npool, oTn_last, sbufs, rbufs, zbuf, zchan = mid
    _ship(nc, len(chunks) - 1, oTn_last, npool, sbufs, rbufs, zchan,
          s_bc, m_bc, chunks)
    npool.release()

    # ---- output projection over own 2048 rows ----
    NF = QS // 128
    with tc.tile_pool(name="fpool", bufs=4) as fpool, \
         tc.tile_pool(name="fpsum", bufs=4, space="PSUM") as fpsum, \
         tc.tile_pool(name="ftpsum", bufs=4, space="PSUM") as ftpsum:
        def f_load(it):
            r_t = fpool.tile([128, C], BF16, name="r_t", tag="r_t")
            nc.sync.dma_start(r_t[:], zbuf[it * 128:(it + 1) * 128, :])
            bank = ftpsum.tile([128, C], BF16, name="f_tr", tag="f_tr")
            for jc in range(3):
                nc.tensor.transpose(bank[:, jc * 128:(jc + 1) * 128],
                                    r_t[:, jc * 128:(jc + 1) * 128], ident[:])
            op_ch = fpool.tile([128, C], BF16, name="op_ch", tag="op_ch")
            # alternate the PSUM->SBUF copy between ACT and DVE so neither
            # engine serializes consecutive iterations
            if it % 2 == 0:
                nc.scalar.copy(op_ch[:], bank[:])
            else:
                nc.vector.tensor_copy(op_ch[:], bank[:])
            return op_ch

        op_q = [f_load(0), f_load(1), f_load(2)]
        for it in range(NF):
            op_ch = op_q.pop(0)
            if it + 3 < NF:
                op_q.append(f_load(it + 3))
            out_ps = fpsum.tile([128, C], F32, name="out_ps", tag="out_ps")
            for jc in range(3):
                nc.tensor.matmul(out_ps[:], op_ch[:, jc * 128:(jc + 1) * 128],
                                 wch("Wp", jc), start=(jc == 0),
                                 stop=(jc == 2))
            o_t = fpool.tile([128, C], F32, name="o_t", tag="o_t")
            nc.vector.tensor_add(o_t[:], out_ps[:], bias_bc[:])
            nc.sync.dma_start(out_d[it * 128:(it + 1) * 128, :], o_t[:])


def _ships(nc, T, chunks):
    return not (getattr(nc, "_tail_local", True) and T == len(chunks) - 1)


def _norm(nc, T, o_ps, e_sum, npool, chunks):
    q0, w = chunks[T]
    e_red = npool.tile([128, 512], F32, name="e_red", tag="e_red")
    nc.gpsimd.partition_all_reduce(e_red[:, 0:w], e_sum[:, 0:w], 128,
                                   bass_isa.ReduceOp.add)
    rec = npool.tile([1, 512], F32, name="rec", tag="rec")
    nc.vector.reciprocal(rec[:, 0:w], e_red[0:1, 0:w])
    rec_b = npool.tile([128, 512], F32, name="rec_b", tag="rec_b")
    nc.gpsimd.partition_broadcast(rec_b[:, 0:w], rec[:, 0:w])
    oTn = [npool.tile([128, 512], BF16, name=f"oTn{cc}", tag=f"oTn{cc}")
           for cc in range(3)]
    for cc in range(3):
        nc.vector.tensor_mul(oTn[cc][:, 0:w], o_ps[cc][:, 0:w],
                             rec_b[:, 0:w])
    return oTn


def _ship(nc, T, oTn, npool, sbufs, rbufs, zchan, s_bc, m_bc, chunks):
    q0, w = chunks[T]
    last = not _ships(nc, T, chunks)
    coef = (s_bc, m_bc)
    if not last:
        # ship peer-destined half (oTn rows 0..192 by Wv-perm)
        sb = sbufs[T]
        nc.sync.dma_start(sb[0:128, :], oTn[0][:, 0:w])
        nc.sync.dma_start(sb[128:CH, :], oTn[1][0:64, 0:w])
        if getattr(nc, "_sim_cc", False):
            for j in range(2):
                nc.sync.dma_start(rbufs[T][j], sb[:])
        else:
            nc.gpsimd.collective_compute(
                "AllGather", mybir.AluOpType.bypass,
                replica_groups=GROUPS,
                ins=[sb[:].opt()], outs=[rbufs[T][:].opt()])
        # place own half + received peer half into zchan at their GLOBAL
        # query columns, selected by exact 0/1 rank masks:
        #   col-block j (global cols j*QS+q0) =
        #       own * (j == my rank) + rb[j] * (j == peer rank)
        rbt = [npool.tile([128, 512], BF16, name=f"rba{j}", tag=f"rba{j}")
               for j in range(2)]
        rbb = [npool.tile([128, 512], BF16, name=f"rbb{j}", tag=f"rbb{j}")
               for j in range(2)]
        for j in range(2):
            # top 64 received rows land on partitions 64..128 so every DVE
            # op below is partition-aligned with the own rows of oTn[1]
            nc.sync.dma_start(rbt[j][64:128, 0:w], rbufs[T][j, 0:64, :])
            nc.sync.dma_start(rbb[j][:, 0:w], rbufs[T][j, 64:CH, :])
        own_a = oTn[1][64:128, 0:w]
        own_b = oTn[2][:, 0:w]
        for j in range(2):
            za = npool.tile([128, 512], BF16, name=f"za{j}", tag=f"za{j}")
            zb = npool.tile([128, 512], BF16, name=f"zb{j}", tag=f"zb{j}")
            ta = npool.tile([128, 512], BF16, name=f"ta{j}", tag=f"ta{j}")
            tb = npool.tile([128, 512], BF16, name=f"tb{j}", tag=f"tb{j}")
            nc.vector.tensor_mul(za[64:128, 0:w], own_a,
                                 coef[j][64:128, 0:w])
            nc.vector.tensor_mul(zb[:, 0:w], own_b, coef[j][:, 0:w])
            nc.vector.tensor_mul(ta[64:128, 0:w], rbt[j][64:128, 0:w],
                                 coef[1 - j][64:128, 0:w])
            nc.vector.tensor_mul(tb[:, 0:w], rbb[j][:, 0:w],
                                 coef[1 - j][:, 0:w])
            nc.vector.tensor_add(za[64:128, 0:w], za[64:128, 0:w],
                                 ta[64:128, 0:w])
            nc.vector.tensor_add(zb[:, 0:w], zb[:, 0:w], tb[:, 0:w])
            csl = slice(j * QS + q0, j * QS + q0 + w)
            nc.sync.dma_start(zchan[0:64, csl], za[64:128, 0:w])
            nc.sync.dma_start(zchan[64:CH, csl], zb[:, 0:w])
    else:
        # tail chunk, fully local on both cores and packed in RANK order by
        # the host (rank 0's tail queries then rank 1's, identical on both
        # cores), so the z writes are fixed slices -- no masks at all
        for j in range(2):
            csl = slice(j * QS + q0, j * QS + q0 + QR)
            nc.sync.dma_start(zchan[0:64, csl],
                              oTn[1][64:128, j * QR:(j + 1) * QR])
            nc.sync.dma_start(zchan[64:CH, csl],
                              oTn[2][:, j * QR:(j + 1) * QR])


def make_in_maps(inputs):
    bf = ml_dtypes.bfloat16
    x = np.asarray(inputs["x"], np.float32)
    t = np.asarray(inputs["t"], np.float32)
    A = (np.asarray(inputs["Wq"], np.float32)
         @ np.asarray(inputs["Wk"], np.float32).T).astype(bf)
    Wv = np.asarray(inputs["Wv"], np.float32)
    # per-core column order: [peer's channel half | own half]
    Wv_p = [np.ascontiguousarray(
        np.concatenate([Wv[:, (1 - g) * CH:(2 - g) * CH],
                        Wv[:, g * CH:(g + 1) * CH]], axis=1)).astype(bf)
        for g in range(2)]
    Wp = np.asarray(inputs["Wp"], np.float32).astype(bf)
    bp = np.asarray(inputs["bp"], np.float32).reshape(1, C)
    xTs = [np.ascontiguousarray(x[b].T).astype(bf) for b in range(B)]
    sels = [np.concatenate([np.full(512, float(g == 0), np.float32),
                            np.full(512, float(g == 1), np.float32)]
                           ).reshape(1, 1024)
            for g in range(2)]
    maps = []
    for p in range(N_CORES):
        b, g = p // 2, p % 2
        maps.append({
            "xT": xTs[b],
            "tT": np.ascontiguousarray(np.concatenate(
                ([t[b, g * QS:(g + 1) * QS - QR],
                  t[b, QS - QR:QS], t[b, 2 * QS - QR:2 * QS]]
                 if TAIL_LOCAL else
                 [t[b, g * QS:(g + 1) * QS],
                  np.zeros((QR, C), np.float32)]),
                axis=0).T).astype(bf),
            "A": A, "Wv": Wv_p[g], "Wp": Wp, "bp": bp, "sel": sels[g],
        })
    return maps


def assemble(results):
    out = np.empty((B, TN, C), np.float32)
    for p in range(N_CORES):
        b, g = p // 2, p % 2
        out[b, g * QS:(g + 1) * QS] = results[p]["out"]
    return out


_NC_CACHE = {}


def _get_nc(repeat=1):
    if repeat not in _NC_CACHE:
        _NC_CACHE[repeat] = build(repeat=repeat)
    return _NC_CACHE[repeat]


def kernel(**inputs) -> np.ndarray:
    nc = _get_nc()
    in_maps = make_in_maps(inputs)
    res = run_bass_kernel_spmd(nc, in_maps, list(range(N_CORES)))
    return assemble(res.results)
